# revision 29
# baseline (speedup 1.0000x reference)
"""KernelConv2D (per-pixel dynamic 5x5 depthwise conv) on 8 TRN2 NeuronCores.

Problem: out[b,c,h,w] = sum_{i,j} x_edgepad[b,c,h+i,w+j] * K[b,c,i,j,h,w]
with input [4,32,128,128] f32 and kernel [4,800,128,128] f32 (800 = 32*25).

Sharding: every (b,c) plane is independent, so flatten to 128 planes and put
the plane index on the SBUF partition axis. Each core takes 16 output ROWS of
all 128 planes (row-sharding). With (h, w) both living in the free dimension,
both conv shifts are constant free-dim offsets -> the 5x5 taps of the input
window are expressed as a single overlapping access pattern, no halo exchange
or partition-shifted copies on device. Host pre-pads the input with edge
replication and slices per-core row bands (incl. 2-row halo).

Per core HBM traffic: K 26.2MB + X 1.35MB + out 1.05MB ~= 28.6MB -> ~80us at
~358 GB/s/core: the memory roofline for this problem. Compute is split so DVE
(products + 9-segment reduce) and GpSimd (16-segment add tree) both stay at or
under the DMA time.
"""

import sys

import numpy as np

sys.path.insert(0, "/opt/trn_rl_repo")

import concourse.bacc as bacc
import concourse.bass as bass
import concourse.tile as tile
from concourse import mybir
from concourse.ap import AP
from concourse.bass_utils import run_bass_kernel_spmd

N_CORES = 8
B, C, H, W, KS = 4, 32, 128, 128, 5
NPLANES = B * C          # 128 -> partition axis
NTAPS = KS * KS          # 25
ROWS_PER_CORE = H // N_CORES   # 16
ROWS_PER_CHUNK = 2
# 1-row chunks at both ends: the leading ones start compute after a minimal
# K prefetch; the trailing ones halve the compute tail after the last K byte.
CHUNK_ROWS = [1, 1, 2, 2, 2, 2, 2, 2, 1, 1]
CHUNK_STARTS = [0, 1, 2, 4, 6, 8, 10, 12, 14, 15]
NCHUNK = len(CHUNK_ROWS)
FDW = ROWS_PER_CHUNK * W                   # max output elems per chunk-partition
XW = W + KS - 1                            # 132 padded row width
XROWS = ROWS_PER_CORE + KS - 1             # 20 rows incl halo
F32 = mybir.dt.float32

# Reduction: the otherwise-idle TensorEngine sums all 25 tap-product segments
# with identity matmuls accumulating into one PSUM bank (exact f32: 1.0*x is
# exact, PSUM accumulation is f32 add). ScalarE evacuates PSUM -> SBUF. DVE
# does only the products; GpSimd stays idle (no shared-SBUF-port contention).

_compiled = None


def _build_program():
    nc = bacc.Bacc(
        "TRN2",
        target_bir_lowering=False,
        debug=False,
        enable_asserts=False,
        num_devices=N_CORES,
    )
    # Host pre-arranges k as [plane][chunk][tap][h2][w] so each chunk load is
    # one contiguous per-partition run (few DMA descriptors, near line rate).
    xd = nc.declare_dram_parameter("x", [NPLANES, XROWS * XW], F32, isOutput=False)
    kd = nc.declare_dram_parameter(
        "k", [NPLANES, NTAPS * ROWS_PER_CORE * W], F32, isOutput=False
    )
    od = nc.declare_dram_parameter("o", [NPLANES, ROWS_PER_CORE * W], F32, isOutput=True)
    ed = nc.declare_dram_parameter("eye", [NPLANES, NPLANES], F32, isOutput=False)

    with tile.TileContext(nc) as tc:
        with (
            tc.tile_pool(name="xpool", bufs=1) as xpool,
            tc.tile_pool(name="epool", bufs=1) as epool,
            tc.tile_pool(name="kpool", bufs=3) as kpool,
            tc.tile_pool(name="ppool", bufs=2) as ppool,
            tc.tile_pool(name="dpool", bufs=2) as dpool,
            tc.tile_pool(name="spool", bufs=3, space="PSUM") as spool,
            tc.tile_pool(name="opool", bufs=3) as opool,
        ):
            # Whole padded input band for this core, resident for the kernel.
            xt = xpool.tile([NPLANES, XROWS * XW], F32)
            nc.sync.dma_start(out=xt[:], in_=xd.ap())
            et = epool.tile([NPLANES, NPLANES], F32)
            nc.sync.dma_start(out=et[:], in_=ed.ap())
            xt_ap = xt[:]
            xt_pdim = xt_ap.ap[0]  # (partition step, 128)

            for ch in range(NCHUNK):
                h0 = CHUNK_STARTS[ch]
                rows = CHUNK_ROWS[ch]
                fdw = rows * W
                kt = kpool.tile([NPLANES, NTAPS * FDW], F32, tag="kt")
                # Two sub-loads per chunk: products for taps 0-9 only gate on
                # the first half (cuts the startup ramp by ~half a chunk load).
                base = NTAPS * W * h0
                nc.sync.dma_start(
                    out=kt[:, 0 : NTAPS * fdw],
                    in_=kd.ap()[:, base : base + NTAPS * fdw],
                )
                pt = ppool.tile([NPLANES, NTAPS * FDW], F32, tag="pt")
                # Products: one op per vertical tap i covers the 5 horizontal
                # taps j as an overlapping strided window of the X band (the
                # DVE ISA caps static patterns at 3 free dims).
                seg = KS * fdw
                for i in range(KS):
                    k_view = kt[:, i * seg : (i + 1) * seg].rearrange(
                        "p (j h w) -> p j h w", j=KS, h=rows, w=W
                    )
                    p_view = pt[:, i * seg : (i + 1) * seg].rearrange(
                        "p (j h w) -> p j h w", j=KS, h=rows, w=W
                    )
                    x_view = AP(
                        xt_ap.tensor,
                        xt_ap.offset + (h0 + i) * XW,
                        [xt_pdim, (1, KS), (XW, rows), (1, W)],
                    )
                    nc.vector.tensor_mul(p_view, k_view, x_view)

                # DVE pre-adds 5 tap pairs in one op (taps 0-4 + 5-9) so the
                # 4-pass fp32 PE only accumulates 20 segments, keeping it
                # under the DMA pace.
                dt = dpool.tile([NPLANES, KS * FDW], F32, tag="dt")
                nc.vector.tensor_add(
                    dt[:, 0 : KS * fdw], pt[:, 0 : KS * fdw], pt[:, KS * fdw : 10 * fdw]
                )

                # TensorE: identity matmuls accumulate the remaining segments
                # into one PSUM bank (exact f32 adds).
                st = spool.tile([NPLANES, FDW], F32, tag="st")
                segs = [pt[:, t * fdw : (t + 1) * fdw] for t in range(10, NTAPS)]
                segs += [dt[:, t * fdw : (t + 1) * fdw] for t in range(KS)]
                for t, s in enumerate(segs):
                    nc.tensor.matmul(
                        st[:, 0:fdw],
                        et[:],
                        s,
                        start=(t == 0),
                        stop=(t == len(segs) - 1),
                    )

                # ScalarE: evacuate PSUM -> SBUF, then store.
                ot = opool.tile([NPLANES, FDW], F32, tag="ot")
                nc.scalar.copy(ot[:, 0:fdw], st[:, 0:fdw])
                # Stores go on the ACT HWDGE ring so a compute-gated store
                # never blocks K loads queued on the sync ring (FIFO/ring).
                nc.scalar.dma_start(
                    out=od.ap()[:, h0 * W : h0 * W + fdw], in_=ot[:, 0:fdw]
                )

    nc.compile()
    return nc


def _get_program():
    global _compiled
    if _compiled is None:
        _compiled = _build_program()
    return _compiled


def _shard_inputs(input: np.ndarray, kernel: np.ndarray):
    x = np.ascontiguousarray(input, dtype=np.float32).reshape(NPLANES, H, W)
    xp = np.pad(x, ((0, 0), (2, 2), (2, 2)), mode="edge")  # [128, 132, 132]
    k = np.ascontiguousarray(kernel, dtype=np.float32).reshape(
        NPLANES, NTAPS, H, W
    )
    eye = np.eye(NPLANES, dtype=np.float32)
    in_maps = []
    for c in range(N_CORES):
        r0 = c * ROWS_PER_CORE
        # [plane][tap][16 rows][w] -> per-chunk [plane][tap][rows][w] blocks,
        # concatenated so each chunk is one contiguous per-plane run.
        ks = k[:, :, r0 : r0 + ROWS_PER_CORE, :]
        blocks = [
            ks[:, :, s : s + n, :].reshape(NPLANES, NTAPS * n * W)
            for s, n in zip(CHUNK_STARTS, CHUNK_ROWS)
        ]
        kc = np.ascontiguousarray(np.concatenate(blocks, axis=1))
        in_maps.append(
            {
                "x": np.ascontiguousarray(
                    xp[:, r0 : r0 + XROWS, :]
                ).reshape(NPLANES, XROWS * XW),
                "k": kc,
                "eye": eye,
            }
        )
    return in_maps


last_results = None  # BassKernelResults of the most recent run (for profiling)


def kernel(input: np.ndarray, kernel: np.ndarray, _trace: bool = False):
    global last_results
    nc = _get_program()
    in_maps = _shard_inputs(input, kernel)
    res = run_bass_kernel_spmd(nc, in_maps, list(range(N_CORES)), trace=_trace)
    last_results = res
    out = np.empty((NPLANES, H, W), dtype=np.float32)
    for c in range(N_CORES):
        out[:, c * ROWS_PER_CORE : (c + 1) * ROWS_PER_CORE, :] = res.results[c][
            "o"
        ].reshape(NPLANES, ROWS_PER_CORE, W)
    return out.reshape(B, C, H, W)


if __name__ == "__main__":
    rng = np.random.default_rng(0)
    inp = rng.standard_normal((B, C, H, W), dtype=np.float32)
    kern = rng.standard_normal((B, C * NTAPS, H, W), dtype=np.float32)
    out = kernel(inp, kern)
    print("ran ok", out.shape, out.dtype)


# revision 30
# speedup vs baseline: 1.0328x; 1.0328x over previous
"""KernelConv2D (per-pixel dynamic 5x5 depthwise conv) on 8 TRN2 NeuronCores.

Problem: out[b,c,h,w] = sum_{i,j} x_edgepad[b,c,h+i,w+j] * K[b,c,i,j,h,w]
with input [4,32,128,128] f32 and kernel [4,800,128,128] f32 (800 = 32*25).

Sharding: every (b,c) plane is independent, so flatten to 128 planes and put
the plane index on the SBUF partition axis. Each core takes 16 output ROWS of
all 128 planes (row-sharding). With (h, w) both living in the free dimension,
both conv shifts are constant free-dim offsets -> the 5x5 taps of the input
window are expressed as a single overlapping access pattern, no halo exchange
or partition-shifted copies on device. Host pre-pads the input with edge
replication and slices per-core row bands (incl. 2-row halo).

Per core HBM traffic: K 26.2MB + X 1.35MB + out 1.05MB ~= 28.6MB -> ~80us at
~358 GB/s/core: the memory roofline for this problem. Compute is split so DVE
(products + 9-segment reduce) and GpSimd (16-segment add tree) both stay at or
under the DMA time.
"""

import sys

import numpy as np

sys.path.insert(0, "/opt/trn_rl_repo")

import concourse.bacc as bacc
import concourse.bass as bass
import concourse.tile as tile
from concourse import mybir
from concourse.ap import AP
from concourse.bass_utils import run_bass_kernel_spmd

N_CORES = 8
B, C, H, W, KS = 4, 32, 128, 128, 5
NPLANES = B * C          # 128 -> partition axis
NTAPS = KS * KS          # 25
ROWS_PER_CORE = H // N_CORES   # 16
ROWS_PER_CHUNK = 2
# 1-row chunks at both ends: the leading ones start compute after a minimal
# K prefetch; the trailing ones halve the compute tail after the last K byte.
CHUNK_ROWS = [1, 1, 2, 2, 2, 2, 2, 2, 1, 1]
CHUNK_STARTS = [0, 1, 2, 4, 6, 8, 10, 12, 14, 15]
NCHUNK = len(CHUNK_ROWS)
FDW = ROWS_PER_CHUNK * W                   # max output elems per chunk-partition
XW = W + KS - 1                            # 132 padded row width
XROWS = ROWS_PER_CORE + KS - 1             # 20 rows incl halo
F32 = mybir.dt.float32

# Reduction: the otherwise-idle TensorEngine sums all 25 tap-product segments
# with identity matmuls accumulating into one PSUM bank (exact f32: 1.0*x is
# exact, PSUM accumulation is f32 add). ScalarE evacuates PSUM -> SBUF. DVE
# does only the products; GpSimd stays idle (no shared-SBUF-port contention).

_compiled = None


def _build_program():
    nc = bacc.Bacc(
        "TRN2",
        target_bir_lowering=False,
        debug=False,
        enable_asserts=False,
        num_devices=N_CORES,
    )
    # Host pre-arranges k as [plane][chunk][tap][h2][w] so each chunk load is
    # one contiguous per-partition run (few DMA descriptors, near line rate).
    xd = nc.declare_dram_parameter("x", [NPLANES, XROWS * XW], F32, isOutput=False)
    kd = nc.declare_dram_parameter(
        "k", [NPLANES, NTAPS * ROWS_PER_CORE * W], F32, isOutput=False
    )
    od = nc.declare_dram_parameter("o", [NPLANES, ROWS_PER_CORE * W], F32, isOutput=True)
    ed = nc.declare_dram_parameter("eye", [NPLANES, NPLANES], F32, isOutput=False)

    with tile.TileContext(nc) as tc:
        with (
            tc.tile_pool(name="xpool", bufs=1) as xpool,
            tc.tile_pool(name="epool", bufs=1) as epool,
            tc.tile_pool(name="kpool", bufs=3) as kpool,
            tc.tile_pool(name="ppool", bufs=2) as ppool,
            tc.tile_pool(name="dpool", bufs=2) as dpool,
            tc.tile_pool(name="spool", bufs=3, space="PSUM") as spool,
            tc.tile_pool(name="opool", bufs=3) as opool,
        ):
            # Whole padded input band for this core, resident for the kernel.
            xt = xpool.tile([NPLANES, XROWS * XW], F32)
            nc.sync.dma_start(out=xt[:], in_=xd.ap())
            et = epool.tile([NPLANES, NPLANES], F32)
            nc.sync.dma_start(out=et[:], in_=ed.ap())
            xt_ap = xt[:]
            xt_pdim = xt_ap.ap[0]  # (partition step, 128)

            for ch in range(NCHUNK):
                h0 = CHUNK_STARTS[ch]
                rows = CHUNK_ROWS[ch]
                fdw = rows * W
                kt = kpool.tile([NPLANES, NTAPS * FDW], F32, tag="kt")
                # Two sub-loads per chunk: products for taps 0-9 only gate on
                # the first half (cuts the startup ramp by ~half a chunk load).
                base = NTAPS * W * h0
                # Two sub-loads per 2-row chunk: products for taps 0-9 only
                # gate on the first half.
                if rows > 1:
                    nc.sync.dma_start(
                        out=kt[:, 0 : 10 * fdw],
                        in_=kd.ap()[:, base : base + 10 * fdw],
                    )
                    nc.sync.dma_start(
                        out=kt[:, 10 * fdw : NTAPS * fdw],
                        in_=kd.ap()[:, base + 10 * fdw : base + NTAPS * fdw],
                    )
                else:
                    nc.sync.dma_start(
                        out=kt[:, 0 : NTAPS * fdw],
                        in_=kd.ap()[:, base : base + NTAPS * fdw],
                    )
                pt = ppool.tile([NPLANES, NTAPS * FDW], F32, tag="pt")
                # Products: one op per vertical tap i covers the 5 horizontal
                # taps j as an overlapping strided window of the X band (the
                # DVE ISA caps static patterns at 3 free dims).
                seg = KS * fdw
                for i in range(KS):
                    k_view = kt[:, i * seg : (i + 1) * seg].rearrange(
                        "p (j h w) -> p j h w", j=KS, h=rows, w=W
                    )
                    p_view = pt[:, i * seg : (i + 1) * seg].rearrange(
                        "p (j h w) -> p j h w", j=KS, h=rows, w=W
                    )
                    x_view = AP(
                        xt_ap.tensor,
                        xt_ap.offset + (h0 + i) * XW,
                        [xt_pdim, (1, KS), (XW, rows), (1, W)],
                    )
                    nc.vector.tensor_mul(p_view, k_view, x_view)

                # DVE pre-adds 5 tap pairs in one op (taps 0-4 + 5-9) so the
                # 4-pass fp32 PE only accumulates 20 segments, keeping it
                # under the DMA pace.
                dt = dpool.tile([NPLANES, KS * FDW], F32, tag="dt")
                nc.vector.tensor_add(
                    dt[:, 0 : KS * fdw], pt[:, 0 : KS * fdw], pt[:, KS * fdw : 10 * fdw]
                )

                # TensorE: identity matmuls accumulate the remaining segments
                # into one PSUM bank (exact f32 adds).
                st = spool.tile([NPLANES, FDW], F32, tag="st")
                segs = [pt[:, t * fdw : (t + 1) * fdw] for t in range(10, NTAPS)]
                segs += [dt[:, t * fdw : (t + 1) * fdw] for t in range(KS)]
                for t, s in enumerate(segs):
                    nc.tensor.matmul(
                        st[:, 0:fdw],
                        et[:],
                        s,
                        start=(t == 0),
                        stop=(t == len(segs) - 1),
                    )

                # ScalarE: evacuate PSUM -> SBUF, then store.
                ot = opool.tile([NPLANES, FDW], F32, tag="ot")
                nc.scalar.copy(ot[:, 0:fdw], st[:, 0:fdw])
                # Stores go on the ACT HWDGE ring so a compute-gated store
                # never blocks K loads queued on the sync ring (FIFO/ring).
                nc.scalar.dma_start(
                    out=od.ap()[:, h0 * W : h0 * W + fdw], in_=ot[:, 0:fdw]
                )

    nc.compile()
    return nc


def _get_program():
    global _compiled
    if _compiled is None:
        _compiled = _build_program()
    return _compiled


def _shard_inputs(input: np.ndarray, kernel: np.ndarray):
    x = np.ascontiguousarray(input, dtype=np.float32).reshape(NPLANES, H, W)
    xp = np.pad(x, ((0, 0), (2, 2), (2, 2)), mode="edge")  # [128, 132, 132]
    k = np.ascontiguousarray(kernel, dtype=np.float32).reshape(
        NPLANES, NTAPS, H, W
    )
    eye = np.eye(NPLANES, dtype=np.float32)
    in_maps = []
    for c in range(N_CORES):
        r0 = c * ROWS_PER_CORE
        # [plane][tap][16 rows][w] -> per-chunk [plane][tap][rows][w] blocks,
        # concatenated so each chunk is one contiguous per-plane run.
        ks = k[:, :, r0 : r0 + ROWS_PER_CORE, :]
        blocks = [
            ks[:, :, s : s + n, :].reshape(NPLANES, NTAPS * n * W)
            for s, n in zip(CHUNK_STARTS, CHUNK_ROWS)
        ]
        kc = np.ascontiguousarray(np.concatenate(blocks, axis=1))
        in_maps.append(
            {
                "x": np.ascontiguousarray(
                    xp[:, r0 : r0 + XROWS, :]
                ).reshape(NPLANES, XROWS * XW),
                "k": kc,
                "eye": eye,
            }
        )
    return in_maps


last_results = None  # BassKernelResults of the most recent run (for profiling)


def kernel(input: np.ndarray, kernel: np.ndarray, _trace: bool = False):
    global last_results
    nc = _get_program()
    in_maps = _shard_inputs(input, kernel)
    res = run_bass_kernel_spmd(nc, in_maps, list(range(N_CORES)), trace=_trace)
    last_results = res
    out = np.empty((NPLANES, H, W), dtype=np.float32)
    for c in range(N_CORES):
        out[:, c * ROWS_PER_CORE : (c + 1) * ROWS_PER_CORE, :] = res.results[c][
            "o"
        ].reshape(NPLANES, ROWS_PER_CORE, W)
    return out.reshape(B, C, H, W)


if __name__ == "__main__":
    rng = np.random.default_rng(0)
    inp = rng.standard_normal((B, C, H, W), dtype=np.float32)
    kern = rng.standard_normal((B, C * NTAPS, H, W), dtype=np.float32)
    out = kernel(inp, kern)
    print("ran ok", out.shape, out.dtype)
